# revision 37
# baseline (speedup 1.0000x reference)
"""MinkowskiGlobalPooling (average=True) segment-mean kernel for 8 trn2 cores.

Full inputs in, full output out. Internally:
  - rows are sharded across 8 cores (500k rows each), then laid out per core
    as 128 SBUF partitions x R rows (tail rows padded with idx=255),
  - host packs a per-core contiguous f32 stream of [64 feats + ones-col] rows
    grouped by chunk, plus a uint8 index sideband (preloaded once),
  - each core builds one-hot masks (mask[p,b] = (idx[p]==b)) on VectorE and
    accumulates per-batch sums+counts via fp32 matmuls into a PSUM tile
    (4 PE column-group strips; last column = counts via the ones column),
  - host sums the 8 per-core partial strips and divides.
"""

import numpy as np


def _ensure_import_path():
    try:
        import concourse.bass  # noqa: F401
    except ImportError:
        import sys

        for p in ("/opt/trn_rl_repo", "/root/.axon_site/_ro/trn_rl_repo"):
            if p not in sys.path:
                sys.path.insert(0, p)


N_CORES = 8
B = 32  # batches
C = 64  # channels
CP1 = C + 1  # channels + ones column
N_TOTAL = 4_000_000
N_CORE = N_TOTAL // N_CORES  # 500_000 real rows per core
P = 128  # SBUF partitions
R = 3920  # rows per partition (128*3920 = 501_760 >= 500_000; tail is padding)
TM = 49  # rows per mask-generation op
# chunk sizes: small lead-in/tail chunks shorten pipeline fill/drain
SCHEDULE = [49, 49, 98] + [147] * 25 + [49]
assert sum(SCHEDULE) == R and all(s % TM == 0 for s in SCHEDULE)
PAD_IDX = 255  # uint8 padding index; matches no batch


def build_program(p=P, schedule=None, tm=TM, fbufs=4, mbufs=4, col_groups=4):
    """Build the per-core Bass program. All cores run the identical program."""
    _ensure_import_path()
    import concourse.mybir as mybir
    from concourse import bacc
    from concourse.tile import TileContext

    f32 = mybir.dt.float32
    u8 = mybir.dt.uint8
    if schedule is None:
        schedule = SCHEDULE
    r = sum(schedule)
    n_mm = r
    assert all(s % tm == 0 for s in schedule) and n_mm % col_groups == 0

    nc = bacc.Bacc()
    stream = nc.dram_tensor("stream", [p * r * CP1], f32, kind="ExternalInput")
    idxu = nc.dram_tensor("idxu", [p * r], u8, kind="ExternalInput")
    iota = nc.dram_tensor("iota", [p, tm * B], f32, kind="ExternalInput")
    out = nc.dram_tensor("out", [col_groups * B, CP1], f32, kind="ExternalOutput")

    with TileContext(nc) as tc:
        with (
            tc.tile_pool(name="const", bufs=1) as cpool,
            tc.tile_pool(name="feats", bufs=fbufs) as fpool,
            tc.tile_pool(name="mask", bufs=mbufs) as mpool,
            tc.tile_pool(name="psum", bufs=1, space="PSUM") as ppool,
            tc.tile_pool(name="outp", bufs=1) as opool,
        ):
            iota_sb = cpool.tile([p, tm * B], f32)
            nc.sync.dma_start(out=iota_sb[:], in_=iota[:, :])
            idx_sb = cpool.tile([p, r], u8)
            nc.sync.dma_start(out=idx_sb[:], in_=idxu[:].rearrange("(p r) -> p r", p=p))

            psum = ppool.tile([col_groups * B, CP1], f32)
            if col_groups > 1:
                # Zero-valued "start" matmuls, one per column-group strip.
                # All real matmuls then accumulate (start=False), making the
                # result independent of the has_written-clear granularity.
                zero_mk = cpool.tile([p, B], f32)
                nc.vector.memset(zero_mk[:], 0.0)
                for g in range(col_groups):
                    nc.tensor.matmul(
                        psum[g * B : (g + 1) * B, :],
                        lhsT=zero_mk[:],
                        rhs=iota_sb[:, :CP1],
                        start=True,
                        stop=False,
                        tile_position=(0, g * B),
                        skip_group_check=True,
                    )
            k = 0
            off = 0  # row offset within a partition
            for j, t in enumerate(schedule):
                ft = fpool.tile([p, t * CP1], f32, tag="ft")
                nc.gpsimd.dma_start(
                    out=ft[:],
                    in_=stream[p * off * CP1 : p * (off + t) * CP1].rearrange(
                        "(p x) -> p x", p=p
                    ),
                )
                for s in range(t // tm):
                    mk = mpool.tile([p, tm * B], f32, tag="mk")
                    nc.vector.tensor_tensor(
                        out=mk[:].rearrange("p (t b) -> p t b", b=B),
                        in0=idx_sb[:, off + s * tm : off + (s + 1) * tm]
                        .unsqueeze(2)
                        .to_broadcast([p, tm, B]),
                        in1=iota_sb[:].rearrange("p (t b) -> p t b", b=B),
                        op=mybir.AluOpType.is_equal,
                    )
                    for ts_ in range(tm):
                        tt = s * tm + ts_
                        g = k % col_groups
                        nc.tensor.matmul(
                            psum[g * B : (g + 1) * B, :],
                            lhsT=mk[:, ts_ * B : (ts_ + 1) * B],
                            rhs=ft[:, tt * CP1 : (tt + 1) * CP1],
                            start=(col_groups == 1 and k == 0),
                            stop=(k >= n_mm - col_groups),
                            tile_position=(0, g * B) if col_groups > 1 else None,
                            skip_group_check=(col_groups > 1),
                        )
                        k += 1
                off += t
            out_sb = opool.tile([col_groups * B, CP1], f32)
            nc.vector.tensor_copy(out=out_sb[:], in_=psum[:])
            nc.sync.dma_start(out=out[:, :], in_=out_sb[:])
    nc.finalize()
    return nc


def host_prep(feats, batch_idx):
    """Build per-core input maps (packed stream layout) from full inputs."""
    feats = np.asarray(feats, dtype=np.float32)
    bi = np.asarray(batch_idx)
    n, c = feats.shape
    assert n == N_TOTAL and c == C, (n, c)

    iota_rep = np.tile(np.arange(B, dtype=np.float32), (P, TM))  # [P, TM*B]
    offs = np.concatenate([[0], np.cumsum(SCHEDULE)])

    in_maps = []
    for m in range(N_CORES):
        sl = slice(m * N_CORE, (m + 1) * N_CORE)
        fpad = np.zeros((P * R, CP1), dtype=np.float32)
        fpad[:N_CORE, :C] = feats[sl]
        fpad[:, C] = 1.0  # ones column (pad rows never selected by any mask)
        fv = fpad.reshape(P, R, CP1)
        ipad = np.full(P * R, PAD_IDX, dtype=np.uint8)
        ipad[:N_CORE] = bi[sl].astype(np.uint8)

        # chunk-major flat layout: chunk j = [p, t_j, CP1] contiguous block
        flat = np.empty(P * R * CP1, dtype=np.float32)
        pos = 0
        for j, t in enumerate(SCHEDULE):
            blk = fv[:, offs[j] : offs[j] + t]  # [P, t, CP1]
            flat[pos : pos + blk.size] = blk.reshape(-1)
            pos += blk.size
        in_maps.append({"stream": flat, "idxu": ipad, "iota": iota_rep})
    return in_maps


_CACHED_NC = None


def get_program():
    global _CACHED_NC
    if _CACHED_NC is None:
        _CACHED_NC = build_program()
    return _CACHED_NC


def run_on_cores(in_maps, trace=False):
    _ensure_import_path()
    from concourse.bass_utils import run_bass_kernel_spmd

    nc = get_program()
    res = run_bass_kernel_spmd(nc, in_maps, list(range(N_CORES)), trace=trace)
    return res


def finalize(per_core_outs):
    acc = np.zeros((B, CP1), dtype=np.float64)
    for o in per_core_outs:
        o = np.asarray(o, dtype=np.float64)
        acc += o.reshape(-1, B, CP1).sum(axis=0)
    sums = acc[:, :C]
    counts = acc[:, C]
    pooled = sums / np.maximum(counts, 1.0)[:, None]
    return pooled.astype(np.float32)


def kernel(feats, batch_idx, num_batches):
    assert int(num_batches) == B
    in_maps = host_prep(feats, batch_idx)
    res = run_on_cores(in_maps)
    return finalize([r["out"] for r in res.results])


# revision 38
# speedup vs baseline: 1.0005x; 1.0005x over previous
"""MinkowskiGlobalPooling (average=True) segment-mean kernel for 8 trn2 cores.

Full inputs in, full output out. Internally:
  - rows are sharded across 8 cores (500k rows each), then laid out per core
    as 128 SBUF partitions x R rows (tail rows padded with idx=255),
  - host packs a per-core contiguous f32 stream of [64 feats + ones-col] rows
    grouped by chunk, plus a uint8 index sideband (preloaded once),
  - each core builds one-hot masks (mask[p,b] = (idx[p]==b)) on VectorE and
    accumulates per-batch sums+counts via fp32 matmuls into a PSUM tile
    (4 PE column-group strips; last column = counts via the ones column),
  - host sums the 8 per-core partial strips and divides.
"""

import numpy as np


def _ensure_import_path():
    try:
        import concourse.bass  # noqa: F401
    except ImportError:
        import sys

        for p in ("/opt/trn_rl_repo", "/root/.axon_site/_ro/trn_rl_repo"):
            if p not in sys.path:
                sys.path.insert(0, p)


N_CORES = 8
B = 32  # batches
C = 64  # channels
CP1 = C + 1  # channels + ones column
N_TOTAL = 4_000_000
N_CORE = N_TOTAL // N_CORES  # 500_000 real rows per core
P = 128  # SBUF partitions
R = 3920  # rows per partition (128*3920 = 501_760 >= 500_000; tail is padding)
TM = 49  # rows per mask-generation op
# chunk sizes: small lead-in/tail chunks shorten pipeline fill/drain
SCHEDULE = [49, 49, 98] + [196] * 18 + [98, 49, 49]
assert sum(SCHEDULE) == R and all(s % TM == 0 for s in SCHEDULE)
PAD_IDX = 255  # uint8 padding index; matches no batch


def build_program(p=P, schedule=None, tm=TM, fbufs=3, mbufs=4, col_groups=4):
    """Build the per-core Bass program. All cores run the identical program."""
    _ensure_import_path()
    import concourse.mybir as mybir
    from concourse import bacc
    from concourse.tile import TileContext

    f32 = mybir.dt.float32
    u8 = mybir.dt.uint8
    if schedule is None:
        schedule = SCHEDULE
    r = sum(schedule)
    n_mm = r
    assert all(s % tm == 0 for s in schedule) and n_mm % col_groups == 0

    nc = bacc.Bacc()
    stream = nc.dram_tensor("stream", [p * r * CP1], f32, kind="ExternalInput")
    idxu = nc.dram_tensor("idxu", [p * r], u8, kind="ExternalInput")
    iota = nc.dram_tensor("iota", [p, tm * B], f32, kind="ExternalInput")
    out = nc.dram_tensor("out", [col_groups * B, CP1], f32, kind="ExternalOutput")

    with TileContext(nc) as tc:
        with (
            tc.tile_pool(name="const", bufs=1) as cpool,
            tc.tile_pool(name="feats", bufs=fbufs) as fpool,
            tc.tile_pool(name="mask", bufs=mbufs) as mpool,
            tc.tile_pool(name="psum", bufs=1, space="PSUM") as ppool,
            tc.tile_pool(name="outp", bufs=1) as opool,
        ):
            iota_sb = cpool.tile([p, tm * B], f32)
            nc.sync.dma_start(out=iota_sb[:], in_=iota[:, :])
            idx_sb = cpool.tile([p, r], u8)
            nc.sync.dma_start(out=idx_sb[:], in_=idxu[:].rearrange("(p r) -> p r", p=p))

            psum = ppool.tile([col_groups * B, CP1], f32)
            if col_groups > 1:
                # Zero-valued "start" matmuls, one per column-group strip.
                # All real matmuls then accumulate (start=False), making the
                # result independent of the has_written-clear granularity.
                zero_mk = cpool.tile([p, B], f32)
                nc.vector.memset(zero_mk[:], 0.0)
                for g in range(col_groups):
                    nc.tensor.matmul(
                        psum[g * B : (g + 1) * B, :],
                        lhsT=zero_mk[:],
                        rhs=iota_sb[:, :CP1],
                        start=True,
                        stop=False,
                        tile_position=(0, g * B),
                        skip_group_check=True,
                    )
            k = 0
            off = 0  # row offset within a partition
            for j, t in enumerate(schedule):
                ft = fpool.tile([p, t * CP1], f32, tag="ft")
                nc.gpsimd.dma_start(
                    out=ft[:],
                    in_=stream[p * off * CP1 : p * (off + t) * CP1].rearrange(
                        "(p x) -> p x", p=p
                    ),
                )
                for s in range(t // tm):
                    mk = mpool.tile([p, tm * B], f32, tag="mk")
                    nc.vector.tensor_tensor(
                        out=mk[:].rearrange("p (t b) -> p t b", b=B),
                        in0=idx_sb[:, off + s * tm : off + (s + 1) * tm]
                        .unsqueeze(2)
                        .to_broadcast([p, tm, B]),
                        in1=iota_sb[:].rearrange("p (t b) -> p t b", b=B),
                        op=mybir.AluOpType.is_equal,
                    )
                    for ts_ in range(tm):
                        tt = s * tm + ts_
                        g = k % col_groups
                        nc.tensor.matmul(
                            psum[g * B : (g + 1) * B, :],
                            lhsT=mk[:, ts_ * B : (ts_ + 1) * B],
                            rhs=ft[:, tt * CP1 : (tt + 1) * CP1],
                            start=(col_groups == 1 and k == 0),
                            stop=(k >= n_mm - col_groups),
                            tile_position=(0, g * B) if col_groups > 1 else None,
                            skip_group_check=(col_groups > 1),
                        )
                        k += 1
                off += t
            out_sb = opool.tile([col_groups * B, CP1], f32)
            nc.vector.tensor_copy(out=out_sb[:], in_=psum[:])
            nc.sync.dma_start(out=out[:, :], in_=out_sb[:])
    nc.finalize()
    return nc


def host_prep(feats, batch_idx):
    """Build per-core input maps (packed stream layout) from full inputs."""
    feats = np.asarray(feats, dtype=np.float32)
    bi = np.asarray(batch_idx)
    n, c = feats.shape
    assert n == N_TOTAL and c == C, (n, c)

    iota_rep = np.tile(np.arange(B, dtype=np.float32), (P, TM))  # [P, TM*B]
    offs = np.concatenate([[0], np.cumsum(SCHEDULE)])

    in_maps = []
    for m in range(N_CORES):
        sl = slice(m * N_CORE, (m + 1) * N_CORE)
        fpad = np.zeros((P * R, CP1), dtype=np.float32)
        fpad[:N_CORE, :C] = feats[sl]
        fpad[:, C] = 1.0  # ones column (pad rows never selected by any mask)
        fv = fpad.reshape(P, R, CP1)
        ipad = np.full(P * R, PAD_IDX, dtype=np.uint8)
        ipad[:N_CORE] = bi[sl].astype(np.uint8)

        # chunk-major flat layout: chunk j = [p, t_j, CP1] contiguous block
        flat = np.empty(P * R * CP1, dtype=np.float32)
        pos = 0
        for j, t in enumerate(SCHEDULE):
            blk = fv[:, offs[j] : offs[j] + t]  # [P, t, CP1]
            flat[pos : pos + blk.size] = blk.reshape(-1)
            pos += blk.size
        in_maps.append({"stream": flat, "idxu": ipad, "iota": iota_rep})
    return in_maps


_CACHED_NC = None


def get_program():
    global _CACHED_NC
    if _CACHED_NC is None:
        _CACHED_NC = build_program()
    return _CACHED_NC


def run_on_cores(in_maps, trace=False):
    _ensure_import_path()
    from concourse.bass_utils import run_bass_kernel_spmd

    nc = get_program()
    res = run_bass_kernel_spmd(nc, in_maps, list(range(N_CORES)), trace=trace)
    return res


def finalize(per_core_outs):
    acc = np.zeros((B, CP1), dtype=np.float64)
    for o in per_core_outs:
        o = np.asarray(o, dtype=np.float64)
        acc += o.reshape(-1, B, CP1).sum(axis=0)
    sums = acc[:, :C]
    counts = acc[:, C]
    pooled = sums / np.maximum(counts, 1.0)[:, None]
    return pooled.astype(np.float32)


def kernel(feats, batch_idx, num_batches):
    assert int(num_batches) == B
    in_maps = host_prep(feats, batch_idx)
    res = run_on_cores(in_maps)
    return finalize([r["out"] for r in res.results])
